# revision 1
# baseline (speedup 1.0000x reference)
"""TRN2 Bass kernel for nn_AttentionCell (BitLinear GQA attention cell).

Sharding (8 cores): data-parallel over batch (2) x tensor-parallel over the
4 KV head-groups (4 query heads each), per the problem's sharding hint. Each
core computes Q/K/V projections, causal softmax attention for its 4 heads,
and a row-parallel partial of the output projection; the host sums the 4
partials per batch (the tensor-parallel all-reduce) and applies the final
RMSNorm row scale.

Precision: fp16 hi/lo pairs (~22-bit mantissa) through x, Q, K and a 2-pass
score matmul (hi*hi with the -rowmax bias folded in via an augmented
contraction row, plus one stacked pass computing both cross terms);
approximate row-max from a hi*hi stats pass reduced on DVE; exp on ACT to
bf16 P; per-head softmax denominators come free from a ones-augmented V
column in the PV matmul; V is also an fp16-pair and is un-normalized by the
per-token inverse scale after the shared K/V projection.
"""

import numpy as np
import ml_dtypes

import concourse.bass as bass
import concourse.bacc as bacc
import concourse.mybir as mybir
import concourse.tile as tile
from concourse.bass_utils import run_bass_kernel_spmd
from concourse.masks import make_identity

f32 = mybir.dt.float32
f16 = mybir.dt.float16
bf16 = mybir.dt.bfloat16

EPS = np.float32(1.1920929e-07)
B, T, D = 2, 2048, 1024
H, HKV, HD = 16, 4, 64
NH = 4            # local (per-core) query heads
LF = NH * HD      # 256 local q features
P = 128
DT = D // P       # 8 d-tiles
CH = 4            # 512-wide token chunks
CW = 512
QB = T // P       # 16 query row blocks
KB = T // P       # 16 key blocks
NEG = -1.0e30

Exp = mybir.ActivationFunctionType.Exp
Square = mybir.ActivationFunctionType.Square
AOp = mybir.AluOpType


def _build():
    nc = bacc.Bacc("TRN2", target_bir_lowering=False, debug=False)

    xh_d = nc.dram_tensor("xh", [D, T], f16, kind="ExternalInput").ap()
    xl_d = nc.dram_tensor("xl", [D, T], f16, kind="ExternalInput").ap()
    std_d = nc.dram_tensor("stdc", [P, KB], f32, kind="ExternalInput").ap()
    wq = nc.dram_tensor("wq", [D, LF], f16, kind="ExternalInput").ap()
    wkv = nc.dram_tensor("wkv", [D, P], f16, kind="ExternalInput").ap()
    wo = nc.dram_tensor("wo", [LF, D], f16, kind="ExternalInput").ap()
    yp = nc.dram_tensor("yp", [T, D], f32, kind="ExternalOutput").ap()
    ssqa = nc.dram_tensor("ssqa", [1, T], f32, kind="ExternalOutput").ap()

    with tile.TileContext(nc) as tc:
        with (
            tc.tile_pool(name="const", bufs=1) as const,
            tc.tile_pool(name="persist", bufs=1) as persist,
        ):
            ident32 = const.tile([P, P], f32, tag="ident32")
            make_identity(nc, ident32[:])
            # stats mask (S layout [row, key]): key > row -> NEG
            mask_s = const.tile([P, P], f32, tag="mask_s")
            nc.gpsimd.memset(mask_s[:], 0.0)
            nc.gpsimd.affine_select(
                out=mask_s[:], in_=mask_s[:],
                compare_op=AOp.is_ge, fill=NEG,
                base=0, pattern=[[-1, P]], channel_multiplier=1,
            )
            # S.T mask [key, row]: row < key -> NEG
            mask_st = const.tile([P, P], f32, tag="mask_st")
            nc.gpsimd.memset(mask_st[:], 0.0)
            nc.gpsimd.affine_select(
                out=mask_st[:], in_=mask_st[:],
                compare_op=AOp.is_ge, fill=NEG,
                base=0, pattern=[[1, P]], channel_multiplier=-1,
            )

            stdc = persist.tile([P, KB], f32, tag="stdc")
            nc.sync.dma_start(out=stdc[:], in_=std_d[:])
            wq_sb = persist.tile([P, DT, LF], f16, tag="wq_sb")
            nc.sync.dma_start(
                out=wq_sb[:], in_=wq.rearrange("(dt p) f -> p dt f", p=P)
            )
            wkv_sb = persist.tile([P, DT, P], f16, tag="wkv_sb")
            nc.sync.dma_start(
                out=wkv_sb[:], in_=wkv.rearrange("(dt p) f -> p dt f", p=P)
            )
            wo_sb = persist.tile([P, 2, D], f16, tag="wo_sb")
            nc.sync.dma_start(
                out=wo_sb[:], in_=wo.rearrange("(ft p) o -> p ft o", p=P)
            )

            # Q: QTaug = [qh; -max], QTs = [ql; qh].  K: KTaug = [kh; ones],
            # KTs = [kh; kl].
            QTaug = [persist.tile([65, T], f16, tag=f"qta{h}", name=f"qta{h}")
                     for h in range(NH)]
            QTs = [persist.tile([P, T], f16, tag=f"qts{h}", name=f"qts{h}")
                   for h in range(NH)]
            KTaug = persist.tile([65, T], f16, tag="kta")
            KTs = persist.tile([P, T], f16, tag="kts")
            nc.gpsimd.memset(KTaug[64:65, :], 1.0)
            nmall = [persist.tile([1, T], f16, tag=f"nm{h}", name=f"nm{h}")
                     for h in range(NH)]
            Vsb = [persist.tile([P, P], bf16, tag=f"v{kb}", name=f"v{kb}")
                   for kb in range(KB)]
            for kb in range(KB):
                nc.gpsimd.memset(Vsb[kb][:, 96:97], 1.0)
            ones1f = persist.tile([1, P], f32, tag="ones1f")
            nc.gpsimd.memset(ones1f[:], 1.0)
            ones1r = persist.tile([1, P], mybir.dt.float32r, tag="ones1r")
            nc.vector.tensor_copy(ones1r[:], ones1f[:])
            onesc = persist.tile([P, 1], f16, tag="onesc")
            nc.gpsimd.memset(onesc[:], 1.0)
            aTh = [persist.tile([P, T], f16, tag=f"ath{ft}", name=f"ath{ft}")
                   for ft in range(2)]
            ssqrow = persist.tile([1, T], f32, tag="ssqrow")

            # ---------------- phase 2: projections ----------------
            with (
                tc.tile_pool(name="ph2", bufs=4) as ph2,
                tc.tile_pool(name="ph2v", bufs=2) as ph2v,
                tc.tile_pool(name="ps_q", bufs=4, space="PSUM") as ps_q,
                tc.tile_pool(name="ps_kv", bufs=2, space="PSUM") as ps_kv,
                tc.tile_pool(name="ps_vt", bufs=2, space="PSUM") as ps_vt,
            ):
                for c in range(CH):
                    cs = slice(c * CW, (c + 1) * CW)
                    psQ = [ps_q.tile([P, CW], f32, tag="psq",
                                     name=f"psq_{c}_{i}") for i in range(2)]
                    psKV = ps_kv.tile([P, CW], f32, tag="pskv")
                    for d in range(DT):
                        th = ph2.tile([P, CW], f16, tag="th")
                        nc.sync.dma_start(out=th[:], in_=xh_d[d * P:(d + 1) * P, cs])
                        tl = ph2.tile([P, CW], f16, tag="tl")
                        nc.sync.dma_start(out=tl[:], in_=xl_d[d * P:(d + 1) * P, cs])
                        first, last = d == 0, d == DT - 1
                        for ft in range(2):
                            wslc = wq_sb[:, d, ft * P:(ft + 1) * P]
                            nc.tensor.matmul(psQ[ft][:], wslc, th[:],
                                             start=first, stop=False)
                            nc.tensor.matmul(psQ[ft][:], wslc, tl[:],
                                             start=False, stop=last)
                        nc.tensor.matmul(psKV[:], wkv_sb[:, d, :], th[:],
                                         start=first, stop=False)
                        nc.tensor.matmul(psKV[:], wkv_sb[:, d, :], tl[:],
                                         start=False, stop=last)
                    for ft in range(2):
                        for sub in range(2):
                            h = 2 * ft + sub
                            pslc = psQ[ft][sub * HD:(sub + 1) * HD, :]
                            nc.scalar.copy(QTaug[h][0:HD, cs], pslc)
                            nc.vector.tensor_copy(QTs[h][HD:P, cs],
                                                  QTaug[h][0:HD, cs])
                            nc.vector.tensor_sub(QTs[h][0:HD, cs], pslc,
                                                 QTaug[h][0:HD, cs])
                    nc.scalar.copy(KTaug[0:HD, cs], psKV[0:HD, :])
                    nc.vector.tensor_copy(KTs[0:HD, cs], KTaug[0:HD, cs])
                    nc.vector.tensor_sub(KTs[HD:P, cs], psKV[0:HD, :],
                                         KTaug[0:HD, cs])
                    vt = ph2v.tile([HD, CW], f32, tag="vt")
                    nc.scalar.copy(vt[:], psKV[HD:P, :])
                    for s4 in range(4):
                        kb = c * 4 + s4
                        pvt = ps_vt.tile([P, HD], f32, tag="pvt")
                        nc.tensor.transpose(pvt[:], vt[:, s4 * P:(s4 + 1) * P],
                                            ident32[0:HD, 0:HD])
                        nc.scalar.mul(Vsb[kb][:, 0:HD], pvt[:],
                                      stdc[:, kb:kb + 1])
                        nc.vector.scalar_tensor_tensor(
                            Vsb[kb][:, HD:96], pvt[:, 0:32],
                            stdc[:, kb:kb + 1], Vsb[kb][:, 0:32],
                            op0=AOp.mult, op1=AOp.subtract)

            # ---------- phases 3-6: attention + O-proj, per row-chunk ----------
            with (
                tc.tile_pool(name="ph3", bufs=4) as ph3,
                tc.tile_pool(name="ptp", bufs=1) as ptp,
                tc.tile_pool(name="ph6", bufs=3) as ph6,
                tc.tile_pool(name="ps_stat", bufs=2, space="PSUM") as ps_stat,
                tc.tile_pool(name="ps_st", bufs=2, space="PSUM") as ps_st,
                tc.tile_pool(name="ps_nm", bufs=1, space="PSUM") as ps_nm,
                tc.tile_pool(name="ps_y", bufs=2, space="PSUM") as ps_y,
            ):
                PTt = [ptp.tile([P, CW], bf16, tag=f"pt{kc}", name=f"pt{kc}")
                       for kc in range(KB)]
                for rc in range(4):               # 512-row chunks
                    rs = slice(rc * CW, (rc + 1) * CW)
                    for h in range(NH):
                        # stats for the 4 row blocks of this chunk
                        mall = ph3.tile([P, 4], f32, tag="mall", bufs=2)
                        for j in range(4):
                            qb = rc * 4 + j
                            nk = (qb + 1) * P
                            nchunks = (nk + CW - 1) // CW
                            mxs = ph3.tile([P, 4], f32, tag="mxs")
                            for ic in range(nchunks):
                                k0 = ic * CW
                                kw = min(CW, nk - k0)
                                pS = ps_stat.tile([P, CW], f32, tag="pstat")
                                nc.tensor.matmul(
                                    pS[:, :kw],
                                    QTaug[h][0:HD, qb * P:(qb + 1) * P],
                                    KTaug[0:HD, k0:k0 + kw],
                                    start=True, stop=True,
                                )
                                if k0 <= qb * P < k0 + kw:
                                    off = qb * P - k0
                                    nc.vector.tensor_add(pS[:, off:off + P],
                                                         pS[:, off:off + P],
                                                         mask_s[:])
                                nc.vector.reduce_max(mxs[:, ic:ic + 1],
                                                     pS[:, :kw],
                                                     axis=mybir.AxisListType.X)
                            nc.vector.reduce_max(mall[:, j:j + 1],
                                                 mxs[:, 0:nchunks],
                                                 axis=mybir.AxisListType.X)
                        for j in range(4):
                            qb = rc * 4 + j
                            pnm = ps_nm.tile([1, P], f32, tag="pnm")
                            nc.tensor.transpose(pnm[:], mall[:, j:j + 1],
                                                ident32[:])
                            nc.scalar.mul(nmall[h][0:1, qb * P:(qb + 1) * P],
                                          pnm[:], -1.0)
                        nc.scalar.copy(QTaug[h][HD:HD + 1, rs],
                                       nmall[h][0:1, rs])

                        # S.T for this row chunk: 2-pass + mask + exp
                        for kc in range((rc + 1) * 4):
                            c0 = max(kc * P, rc * CW)
                            cw_ = (rc + 1) * CW - c0
                            kslc = slice(kc * P, (kc + 1) * P)
                            cslc = slice(c0, c0 + cw_)
                            lo = c0 - rc * CW
                            pST = ps_st.tile([P, CW], f32, tag="pst")
                            nc.tensor.matmul(pST[:, lo:lo + cw_], KTs[:, kslc],
                                             QTs[h][:, cslc],
                                             start=True, stop=False)
                            nc.tensor.matmul(pST[:, lo:lo + cw_],
                                             KTaug[0:HD + 1, kslc],
                                             QTaug[h][0:HD + 1, cslc],
                                             start=False, stop=True)
                            if c0 == kc * P:
                                nc.vector.tensor_add(pST[:, lo:lo + P],
                                                     pST[:, lo:lo + P],
                                                     mask_st[:])
                            nc.scalar.activation(PTt[kc][:, lo:lo + cw_],
                                                 pST[:, lo:lo + cw_], Exp)

                        # transposed PV: psO rows = [PVh | PVl(0:62) | denom]
                        psO = ps_st.tile([P, CW], f32, tag="pso", bufs=1)
                        for kc in range((rc + 1) * 4):
                            c0 = max(kc * P, rc * CW)
                            cw_ = (rc + 1) * CW - c0
                            lo = c0 - rc * CW
                            nc.tensor.matmul(
                                psO[0:97, lo:lo + cw_], Vsb[kc][:, 0:97],
                                PTt[kc][:, lo:lo + cw_],
                                start=(kc == 0), stop=(kc == (rc + 1) * 4 - 1),
                            )
                        # normalize columns by 1/denom and build fp16 aT
                        rd32 = ph3.tile([1, CW], f32, tag="rd32")
                        nc.vector.reciprocal(rd32[:], psO[96:97, :])
                        rdr = ph3.tile([1, CW], mybir.dt.float32r, tag="rdr")
                        nc.vector.tensor_copy(rdr[:], rd32[:])
                        dbc = ps_y.tile([P, CW], f32, tag="psy",
                                        name=f"dbc_{rc}_{h}")
                        nc.tensor.matmul(dbc[:], ones1r[:], rdr[:],
                                         start=True, stop=True)
                        au = ph3.tile([HD, CW], f32, tag="au")
                        nc.scalar.copy(au[:], psO[0:HD, :])
                        nc.vector.scalar_tensor_tensor(
                            au[0:32, :], psO[HD:96, :], 1.0,
                            au[0:32, :], op0=AOp.mult, op1=AOp.add)
                        ft, sub = h // 2, h % 2
                        nc.vector.tensor_mul(
                            aTh[ft][sub * HD:(sub + 1) * HD, rs],
                            au[:], dbc[0:HD, :])

                    # ssq of the normalized attn rows (partial, this core)
                    psq = ps_y.tile([1, CW], f32, tag="psy", name=f"psq_{rc}")
                    for ft in range(2):
                        sqt = ph6.tile([P, CW], f16, tag="sqt")
                        nc.vector.tensor_mul(sqt[:], aTh[ft][:, rs],
                                             aTh[ft][:, rs])
                        nc.tensor.matmul(psq[:], onesc[:], sqt[:],
                                         start=(ft == 0), stop=(ft == 1))
                    nc.vector.tensor_copy(ssqrow[0:1, rs], psq[:])

                    # O-projection for this row chunk (1-pass fp16)
                    for j in range(4):
                        qb = rc * 4 + j
                        qs = slice(qb * P, (qb + 1) * P)
                        for oc in range(2):
                            os_ = slice(oc * CW, (oc + 1) * CW)
                            psY = ps_y.tile([P, CW], f32, tag="psy",
                                            name=f"psY_{qb}_{oc}")
                            for ft in range(2):
                                nc.tensor.matmul(psY[:], aTh[ft][:, qs],
                                                 wo_sb[:, ft, os_],
                                                 start=(ft == 0),
                                                 stop=(ft == 1))
                            ysb = ph6.tile([P, CW], f32, tag="ysb")
                            nc.scalar.copy(ysb[:], psY[:])
                            nc.sync.dma_start(out=yp[qs, os_], in_=ysb[:])
                nc.sync.dma_start(out=ssqa[:], in_=ssqrow[:])

    nc.finalize()
    return nc


def _ternary(w):
    th = np.abs(w).mean(dtype=np.float64)
    return (np.sign(w) * (np.abs(w) > th)).astype(np.float32)


_CACHE = {}


def kernel(x, q_w, q_g, k_w, k_g, v_w, o_w, o_g, qk_gain):
    x = np.asarray(x, np.float32)
    wq_eff = (_ternary(np.asarray(q_w)) * np.asarray(q_g)[None, :]
              * np.float32(qk_gain) / np.float32(np.sqrt(np.float32(HD))))
    wk_eff = _ternary(np.asarray(k_w)) * np.asarray(k_g)[None, :]
    wo_eff = _ternary(np.asarray(o_w)) * np.asarray(o_g)[None, :]
    wqT = np.ascontiguousarray(wq_eff.T).astype(np.float16)      # [D, H*HD]
    wkT = wk_eff.T.astype(np.float16)                            # [D, HKV*HD]
    wvT = np.asarray(v_w, np.float32).T.astype(np.float16)
    woT = np.ascontiguousarray(wo_eff.T).astype(np.float16)      # [D, D]

    # per-token rms scales (host); x_hat = x * r, V un-normalized on device
    # by std = 1/r
    xs = x.astype(np.float64)
    ssq = (xs * xs).mean(-1) + np.float64(EPS)
    r = (1.0 / np.sqrt(ssq)).astype(np.float32)                  # [B, T]
    std = np.sqrt(ssq).astype(np.float32)                        # [B, T]
    xn = (x * r[:, :, None]).astype(np.float32)

    if "nc" not in _CACHE:
        _CACHE["nc"] = _build()
    nc = _CACHE["nc"]

    in_maps = []
    for core in range(8):
        b, g = divmod(core, 4)
        xnT = np.ascontiguousarray(xn[b].T)                      # [D, T] f32
        xh = xnT.astype(np.float16)
        xl = (xnT - xh.astype(np.float32)).astype(np.float16)
        wkv_c = np.concatenate(
            [wkT[:, g * HD:(g + 1) * HD], wvT[:, g * HD:(g + 1) * HD]], axis=1)
        in_maps.append({
            "xh": xh,
            "xl": xl,
            "stdc": np.ascontiguousarray(std[b].reshape(KB, P).T),
            "wq": np.ascontiguousarray(wqT[:, g * LF:(g + 1) * LF]),
            "wkv": np.ascontiguousarray(wkv_c),
            "wo": np.ascontiguousarray(woT[g * LF:(g + 1) * LF, :]),
        })
    _CACHE["in_maps"] = in_maps
    res = run_bass_kernel_spmd(nc, in_maps, list(range(8)))

    out = np.empty((B, T, D), np.float32)
    for b in range(B):
        ssq_a = np.zeros((T,), np.float32)
        ysum = np.zeros((T, D), np.float32)
        for g in range(4):
            rr = res.results[b * 4 + g]
            ysum += rr["yp"]
            ssq_a += rr["ssqa"][0]
        ro = 1.0 / np.sqrt(ssq_a / np.float32(D) + EPS)
        out[b] = ysum * ro[:, None]
    return out


if __name__ == "__main__":
    data = np.load("/root/problem/inputs.npz")
    out = kernel(**{k: data[k] for k in data.files})
    ref = np.load("/root/problem/ref_out.npy")
    d = out.astype(np.float64) - ref.astype(np.float64)
    rv = (d * d).sum() / (ref.astype(np.float64) ** 2).sum()
    print("resid_var=%.3e relerr=%.3e absmax=%.3g" %
          (rv, np.sqrt(rv), np.abs(d).max()))



# revision 2
# speedup vs baseline: 1.2905x; 1.2905x over previous
"""TRN2 Bass kernel for nn_AttentionCell (BitLinear GQA attention cell).

Sharding (8 cores): data-parallel over batch (2) x tensor-parallel over the
4 KV head-groups (4 query heads each). Each core computes Q/K/V projections,
causal softmax attention for its 4 heads, and a row-parallel partial of the
output projection; the host sums the 4 partials per batch and applies the
final RMSNorm row scale.

Engine rebalance vs the first working version (cost-model-driven):
 - causal masks applied as accumulating identity matmuls into PSUM
   (f16 -60000 fill) instead of DVE tensor adds
 - row-max stats decomposed into per-query-block units kept in a gated
   work queue: units weave into the projection d-loop as soon as their
   K/Q chunks exist, and dribble between the S.T key-block iterations of
   earlier row chunks, so the DVE reduces overlap PE matmul streams
 - -max enters the S.T aug row via one [128,4] transpose + ACT mul +
   SBUF-SBUF DMA reshape (batched per head/row-chunk)
 - V stored bf16 (no lo-correction); 1/denom computed f16 on DVE and
   broadcast via a rank-1 f16 matmul; PSUM evacuation split ACT/DVE by
   row-chunk to balance the exp-heavy tail
 - x loaded into persistent SBUF tiles via the Pool SWDGE queue
"""

import numpy as np
import ml_dtypes

import concourse.bass as bass
import concourse.bacc as bacc
import concourse.mybir as mybir
import concourse.tile as tile
from concourse.bass_utils import run_bass_kernel_spmd
from concourse.masks import make_identity

f32 = mybir.dt.float32
f16 = mybir.dt.float16
bf16 = mybir.dt.bfloat16

EPS = np.float32(1.1920929e-07)
B, T, D = 2, 2048, 1024
H, HKV, HD = 16, 4, 64
NH = 4            # local (per-core) query heads
LF = NH * HD      # 256 local q features
P = 128
DT = D // P       # 8 d-tiles
CH = 4            # 512-wide token chunks
CW = 512
QB = T // P       # 16 query row blocks
KB = T // P       # 16 key blocks
NEG = -1.0e30

Exp = mybir.ActivationFunctionType.Exp
AOp = mybir.AluOpType


def _build():
    nc = bacc.Bacc("TRN2", target_bir_lowering=False, debug=False)

    xh_d = nc.dram_tensor("xh", [D, T], f16, kind="ExternalInput").ap()
    xl_d = nc.dram_tensor("xl", [D, T], f16, kind="ExternalInput").ap()
    std_d = nc.dram_tensor("stdc", [P, KB], f32, kind="ExternalInput").ap()
    wq = nc.dram_tensor("wq", [D, LF], f16, kind="ExternalInput").ap()
    wkv = nc.dram_tensor("wkv", [D, P], f16, kind="ExternalInput").ap()
    wo = nc.dram_tensor("wo", [LF, D], f16, kind="ExternalInput").ap()
    yp = nc.dram_tensor("yp", [T, D], f32, kind="ExternalOutput").ap()
    ssqa = nc.dram_tensor("ssqa", [1, T], f32, kind="ExternalOutput").ap()

    with tile.TileContext(nc) as tc:
        with (
            tc.tile_pool(name="const", bufs=1) as const,
            tc.tile_pool(name="persist", bufs=1) as persist,
            tc.tile_pool(name="phm", bufs=8) as phm,
        ):
            ident32 = const.tile([P, P], f32, tag="ident32")
            make_identity(nc, ident32[:])
            identh = const.tile([P, P], f16, tag="identh")
            nc.vector.tensor_copy(identh[:], ident32[:])
            # causal masks applied via an accumulating identity matmul into
            # PSUM; -60000 is f16-representable and drives exp to zero.
            MNEG = -60000.0
            # stats mask (S layout [row, key]): key > row -> MNEG
            mask_s = const.tile([P, P], f16, tag="mask_s")
            nc.gpsimd.memset(mask_s[:], 0.0)
            nc.gpsimd.affine_select(
                out=mask_s[:], in_=mask_s[:],
                compare_op=AOp.is_ge, fill=MNEG,
                base=0, pattern=[[-1, P]], channel_multiplier=1,
            )
            # S.T mask [key, row]: row < key -> MNEG
            mask_st = const.tile([P, P], f16, tag="mask_st")
            nc.gpsimd.memset(mask_st[:], 0.0)
            nc.gpsimd.affine_select(
                out=mask_st[:], in_=mask_st[:],
                compare_op=AOp.is_ge, fill=MNEG,
                base=0, pattern=[[1, P]], channel_multiplier=-1,
            )

            stdc = persist.tile([P, KB], f32, tag="stdc")
            nc.sync.dma_start(out=stdc[:], in_=std_d[:])
            wq_sb = persist.tile([P, DT, LF], f16, tag="wq_sb")
            nc.sync.dma_start(
                out=wq_sb[:], in_=wq.rearrange("(dt p) f -> p dt f", p=P)
            )
            wkv_sb = persist.tile([P, DT, P], f16, tag="wkv_sb")
            nc.sync.dma_start(
                out=wkv_sb[:], in_=wkv.rearrange("(dt p) f -> p dt f", p=P)
            )
            wo_sb = persist.tile([P, 2, D], f16, tag="wo_sb")

            # Q: QTaug = [qh; -max], QTs = [ql; qh].  K: KTaug = [kh; ones],
            # KTs = [kh; kl].
            QTaug = [persist.tile([65, T], f16, tag=f"qta{h}", name=f"qta{h}")
                     for h in range(NH)]
            QTs = [persist.tile([P, T], f16, tag=f"qts{h}", name=f"qts{h}")
                   for h in range(NH)]
            KTaug = persist.tile([65, T], f16, tag="kta")
            KTs = persist.tile([P, T], f16, tag="kts")
            nc.gpsimd.memset(KTaug[64:65, :], 1.0)
            Vsb = [persist.tile([P, HD + 1], bf16, tag=f"v{kb}", name=f"v{kb}")
                   for kb in range(KB)]
            for kb in range(KB):
                nc.gpsimd.memset(Vsb[kb][:, HD:HD + 1], 1.0)
            ones1h = persist.tile([1, P], f16, tag="ones1h")
            nc.gpsimd.memset(ones1h[:], 1.0)
            onesc = persist.tile([P, 1], f16, tag="onesc")
            nc.gpsimd.memset(onesc[:], 1.0)
            aTh = [persist.tile([P, T], f16, tag=f"ath{ft}", name=f"ath{ft}")
                   for ft in range(2)]
            ssqrow = persist.tile([1, T], f32, tag="ssqrow")

            # full-T x tiles, loaded chunk-major so chunk 0 is ready first
            xhs = persist.tile([P, DT, T], f16, tag="xhs")
            xls = persist.tile([P, DT, T], f16, tag="xls")
            for c in range(CH):
                cs = slice(c * CW, (c + 1) * CW)
                for d in range(DT):
                    nc.gpsimd.dma_start(out=xhs[:, d, cs],
                                        in_=xh_d[d * P:(d + 1) * P, cs])
                    nc.gpsimd.dma_start(out=xls[:, d, cs],
                                        in_=xl_d[d * P:(d + 1) * P, cs])
            nc.sync.dma_start(
                out=wo_sb[:], in_=wo.rearrange("(ft p) o -> p ft o", p=P)
            )

            with tc.tile_pool(name="ps_stat", bufs=2,
                              space="PSUM") as ps_stat:
                mall = {}
                queue = []   # gated stats/nm work queue: (rc, closure)

                def filler(max_rc, budget=1):
                    while budget > 0 and queue and queue[0][0] <= max_rc:
                        queue.pop(0)[1]()
                        budget -= 1

                def drain_for(rc):
                    idxs = [i for i, e in enumerate(queue) if e[0] == rc]
                    if idxs:
                        for _ in range(idxs[-1] + 1):
                            queue.pop(0)[1]()

                def _stats_phase2():
                    # ---------------- phase 2: projections ----------------
                    with (
                        tc.tile_pool(name="ph2v", bufs=2) as ph2v,
                        tc.tile_pool(name="ps_q", bufs=3, space="PSUM") as ps_q,
                        tc.tile_pool(name="ps_kv", bufs=1, space="PSUM") as ps_kv,
                        tc.tile_pool(name="ps_vt", bufs=2, space="PSUM") as ps_vt,
                    ):
                        for c in range(CH):
                            cs = slice(c * CW, (c + 1) * CW)
                            psQ = [ps_q.tile([P, CW], f32, tag="psq",
                                             name=f"psq_{c}_{i}")
                                   for i in range(2)]
                            psKV = ps_kv.tile([P, CW], f32, tag="pskv")
                            for d in range(DT):
                                th = xhs[:, d, cs]
                                tl = xls[:, d, cs]
                                first, last = d == 0, d == DT - 1
                                for ft in range(2):
                                    wslc = wq_sb[:, d, ft * P:(ft + 1) * P]
                                    nc.tensor.matmul(psQ[ft][:], wslc, th,
                                                     start=first, stop=False)
                                    nc.tensor.matmul(psQ[ft][:], wslc, tl,
                                                     start=False, stop=last)
                                nc.tensor.matmul(psKV[:], wkv_sb[:, d, :], th,
                                                 start=first, stop=False)
                                nc.tensor.matmul(psKV[:], wkv_sb[:, d, :], tl,
                                                 start=False, stop=last)
                                filler(c - 1, budget=3)
                            for ft in range(2):
                                for sub in range(2):
                                    h = 2 * ft + sub
                                    pslc = psQ[ft][sub * HD:(sub + 1) * HD, :]
                                    nc.scalar.copy(QTaug[h][0:HD, cs], pslc)
                                    nc.gpsimd.tensor_copy(QTs[h][HD:P, cs],
                                                          QTaug[h][0:HD, cs])
                                    nc.vector.tensor_sub(QTs[h][0:HD, cs],
                                                         pslc,
                                                         QTaug[h][0:HD, cs])
                            nc.scalar.copy(KTaug[0:HD, cs], psKV[0:HD, :])
                            nc.gpsimd.tensor_copy(KTs[0:HD, cs],
                                                  KTaug[0:HD, cs])
                            nc.vector.tensor_sub(KTs[HD:P, cs], psKV[0:HD, :],
                                                 KTaug[0:HD, cs])
                            vt = ph2v.tile([HD, CW], f32, tag="vt")
                            nc.scalar.copy(vt[:], psKV[HD:P, :])
                            for s4 in range(4):
                                kb = c * 4 + s4
                                pvt = ps_vt.tile([P, HD], f32, tag="pvt")
                                nc.tensor.transpose(
                                    pvt[:], vt[:, s4 * P:(s4 + 1) * P],
                                    ident32[0:HD, 0:HD])
                                nc.scalar.mul(Vsb[kb][:, 0:HD], pvt[:],
                                              stdc[:, kb:kb + 1])

                _phase3_pools = (
                    tc.tile_pool(name="ph3", bufs=4),
                    tc.tile_pool(name="ptp", bufs=1),
                    tc.tile_pool(name="ph6", bufs=3),
                    tc.tile_pool(name="ps_st", bufs=3, space="PSUM"),
                    tc.tile_pool(name="ps_y", bufs=2, space="PSUM"),
                )

                def emit_stat_unit(h, qb):
                    """Row-max for query block qb, head h: chunked hi*hi
                    matmuls + reduces; causal diag mask fused into a final
                    tensor_tensor_reduce with init-value chaining."""
                    rc, j = divmod(qb, 4)
                    if j == 0:
                        mall[(h, rc)] = phm.tile([P, 4], f32, tag="mall",
                                                 name=f"mall_{h}_{rc}")
                    ml = mall[(h, rc)]
                    ds = qb * P             # diag start
                    nk = ds + P
                    ts = []
                    for ic in range((nk + CW - 1) // CW):
                        lo = ic * CW
                        w = min(CW, nk - lo)
                        pS = ps_stat.tile([P, CW], f32, tag="pstat",
                                          name=f"pstat_{h}_{qb}_{ic}")
                        has_diag = lo + w > ds
                        nc.tensor.matmul(
                            pS[:, 0:w],
                            QTaug[h][0:HD, qb * P:(qb + 1) * P],
                            KTaug[0:HD, lo:lo + w],
                            start=True, stop=not has_diag,
                        )
                        if has_diag:
                            # causal mask folded in as an identity matmul
                            w0 = ds - lo
                            nc.tensor.matmul(pS[:, w0:w0 + P], identh[:],
                                             mask_s[:], start=False,
                                             stop=True)
                        dst = (ml[:, j:j + 1]
                               if has_diag and not ts and (ic == 0)
                               else None)
                        t = phm.tile([P, 1], f32, tag="tcol",
                                     name=f"t_{h}_{qb}_{ic}")
                        nc.vector.tensor_reduce(
                            out=dst if dst is not None else t[:],
                            in_=pS[:, 0:w],
                            axis=mybir.AxisListType.X, op=AOp.max)
                        if dst is None:
                            ts.append(t)
                    if ts:
                        if len(ts) == 1:
                            nc.vector.tensor_max(ml[:, j:j + 1], ts[0][:],
                                                 ts[0][:])
                        else:
                            while len(ts) > 2:
                                nc.vector.tensor_max(ts[0][:], ts[0][:],
                                                     ts[1][:])
                                ts.pop(1)
                            nc.vector.tensor_max(ml[:, j:j + 1], ts[0][:],
                                                 ts[1][:])

                def emit_nm(h, rc):
                    # -max into QTaug aug row [1, CW] via transpose + DMA
                    rs = slice(rc * CW, (rc + 1) * CW)
                    pnm = ps_y.tile([4, P], f32, tag="psy",
                                    name=f"pnm_{h}_{rc}")
                    nc.tensor.transpose(pnm[:], mall[(h, rc)][:], ident32[:])
                    nm4 = phm.tile([4, P], f16, tag="nm4",
                                   name=f"nm4_{h}_{rc}")
                    nc.scalar.mul(nm4[:], pnm[:], -1.0)
                    nc.sync.dma_start(out=QTaug[h][HD:HD + 1, rs], in_=nm4[:])

                # dribble queue: stats/nm for row-chunk rc+1 are emitted
                # between the attention key-block iterations of row-chunk rc
                queue = []
                for rc in range(1, 4):
                    for h in range(NH):
                        for j in range(4):
                            queue.append((lambda hh=h, qq=rc * 4 + j:
                                          emit_stat_unit(hh, qq)))
                        queue.append((lambda hh=h, rr=rc: emit_nm(hh, rr)))

                def filler():
                    if queue:
                        queue.pop(0)()

                def emit_attn(h, rc):
                    rs = slice(rc * CW, (rc + 1) * CW)
                    ft, sub = h // 2, h % 2
                    # S.T for this row chunk: 2-pass + exp (+ Pool diag zero)
                    for kc in range((rc + 1) * 4):
                        c0 = max(kc * P, rc * CW)
                        cw_ = (rc + 1) * CW - c0
                        kslc = slice(kc * P, (kc + 1) * P)
                        cslc = slice(c0, c0 + cw_)
                        lo = c0 - rc * CW
                        pST = ps_st.tile([P, CW], f32, tag="pst",
                                         name=f"pst_{h}_{rc}_{kc}")
                        nc.tensor.matmul(pST[:, lo:lo + cw_], KTs[:, kslc],
                                         QTs[h][:, cslc],
                                         start=True, stop=False)
                        diag_here = c0 == kc * P
                        nc.tensor.matmul(pST[:, lo:lo + cw_],
                                         KTaug[0:HD + 1, kslc],
                                         QTaug[h][0:HD + 1, cslc],
                                         start=False, stop=not diag_here)
                        if diag_here:
                            # causal mask folded in as an identity matmul
                            # (keeps exp inputs finite for the device)
                            nc.tensor.matmul(pST[:, lo:lo + P], identh[:],
                                             mask_st[:], start=False,
                                             stop=True)
                        nc.scalar.activation(PTt[kc][:, lo:lo + cw_],
                                             pST[:, lo:lo + cw_], Exp)
                        filler()

                    # transposed PV: psO rows = [PVh | denom]
                    psO = ps_st.tile([P, CW], f32, tag="pso", bufs=1,
                                     name=f"pso_{h}_{rc}")
                    for kc in range((rc + 1) * 4):
                        c0 = max(kc * P, rc * CW)
                        cw_ = (rc + 1) * CW - c0
                        lo = c0 - rc * CW
                        nc.tensor.matmul(
                            psO[0:HD + 1, lo:lo + cw_], Vsb[kc][:, 0:HD + 1],
                            PTt[kc][:, lo:lo + cw_],
                            start=(kc == 0), stop=(kc == (rc + 1) * 4 - 1),
                        )
                    filler()
                    # 1/denom (f16) -> broadcast to [P, CW] via rank-1 matmul
                    rd16 = ph3.tile([1, CW], f16, tag="rd16")
                    with nc.allow_low_precision(reason="1/denom broadcast; f16 rel err 2^-11 is within budget"):
                        nc.vector.reciprocal(rd16[:], psO[HD:HD + 1, :])
                    dbc = ps_y.tile([P, CW], f32, tag="psy",
                                    name=f"dbc_{rc}_{h}")
                    nc.tensor.matmul(dbc[:], ones1h[:], rd16[:],
                                     start=True, stop=True)
                    au = ph3.tile([HD, CW], f32, tag="au")
                    if rc < 2:
                        nc.scalar.copy(au[:], psO[0:HD, :])
                    else:
                        # ACT is exp-saturated in late row chunks; DVE idles
                        nc.vector.tensor_copy(au[:], psO[0:HD, :])
                    nc.vector.tensor_mul(
                        aTh[ft][sub * HD:(sub + 1) * HD, rs],
                        au[:], dbc[0:HD, :])
                    filler()

                def emit_ssq_oproj(rc):
                    rs = slice(rc * CW, (rc + 1) * CW)
                    # ssq of the normalized attn rows (partial, this core)
                    psq = ps_y.tile([1, CW], f32, tag="psy", name=f"psq_{rc}")
                    for ft in range(2):
                        sqt = ph6.tile([P, CW], f16, tag="sqt")
                        nc.gpsimd.tensor_mul(sqt[:], aTh[ft][:, rs],
                                             aTh[ft][:, rs])
                        nc.tensor.matmul(psq[:], onesc[:], sqt[:],
                                         start=(ft == 0), stop=(ft == 1))
                    if rc < 2:
                        nc.scalar.copy(ssqrow[0:1, rs], psq[:])
                    else:
                        nc.vector.tensor_copy(ssqrow[0:1, rs], psq[:])

                    # O-projection for this row chunk (1-pass fp16)
                    for j in range(4):
                        qb = rc * 4 + j
                        qs = slice(qb * P, (qb + 1) * P)
                        for oc in range(2):
                            os_ = slice(oc * CW, (oc + 1) * CW)
                            psY = ps_y.tile([P, CW], f32, tag="psy",
                                            name=f"psY_{qb}_{oc}")
                            for ft in range(2):
                                nc.tensor.matmul(psY[:], aTh[ft][:, qs],
                                                 wo_sb[:, ft, os_],
                                                 start=(ft == 0),
                                                 stop=(ft == 1))
                            ysb = ph6.tile([P, CW], f32, tag="ysb")
                            if rc < 2:
                                nc.scalar.copy(ysb[:], psY[:])
                            else:
                                nc.vector.tensor_copy(ysb[:], psY[:])
                            nc.sync.dma_start(out=yp[qs, os_], in_=ysb[:])
                        filler()

                # prologue: stats + aug rows for row-chunk 0
                for h in range(NH):
                    for j in range(4):
                        emit_stat_unit(h, j)
                    emit_nm(h, 0)
                for rc in range(4):
                    for h in range(NH):
                        emit_attn(h, rc)
                    emit_ssq_oproj(rc)
                while queue:
                    filler()
                nc.sync.dma_start(out=ssqa[:], in_=ssqrow[:])

    nc.finalize()
    return nc


def _ternary(w):
    th = np.abs(w).mean(dtype=np.float64)
    return (np.sign(w) * (np.abs(w) > th)).astype(np.float32)


_CACHE = {}


def kernel(x, q_w, q_g, k_w, k_g, v_w, o_w, o_g, qk_gain):
    x = np.asarray(x, np.float32)
    wq_eff = (_ternary(np.asarray(q_w)) * np.asarray(q_g)[None, :]
              * np.float32(qk_gain) / np.float32(np.sqrt(np.float32(HD))))
    wk_eff = _ternary(np.asarray(k_w)) * np.asarray(k_g)[None, :]
    wo_eff = _ternary(np.asarray(o_w)) * np.asarray(o_g)[None, :]
    wqT = np.ascontiguousarray(wq_eff.T).astype(np.float16)      # [D, H*HD]
    wkT = wk_eff.T.astype(np.float16)                            # [D, HKV*HD]
    wvT = np.asarray(v_w, np.float32).T.astype(np.float16)
    woT = np.ascontiguousarray(wo_eff.T).astype(np.float16)      # [D, D]

    # per-token rms scales (host); x_hat = x * r, V un-normalized on device
    # by std = 1/r
    xs = x.astype(np.float64)
    ssq = (xs * xs).mean(-1) + np.float64(EPS)
    r = (1.0 / np.sqrt(ssq)).astype(np.float32)                  # [B, T]
    std = np.sqrt(ssq).astype(np.float32)                        # [B, T]
    xn = (x * r[:, :, None]).astype(np.float32)

    if "nc" not in _CACHE:
        _CACHE["nc"] = _build()
    nc = _CACHE["nc"]

    in_maps = []
    for core in range(8):
        b, g = divmod(core, 4)
        xnT = np.ascontiguousarray(xn[b].T)                      # [D, T] f32
        xh = xnT.astype(np.float16)
        xl = (xnT - xh.astype(np.float32)).astype(np.float16)
        wkv_c = np.concatenate(
            [wkT[:, g * HD:(g + 1) * HD], wvT[:, g * HD:(g + 1) * HD]], axis=1)
        in_maps.append({
            "xh": xh,
            "xl": xl,
            "stdc": np.ascontiguousarray(std[b].reshape(KB, P).T),
            "wq": np.ascontiguousarray(wqT[:, g * LF:(g + 1) * LF]),
            "wkv": np.ascontiguousarray(wkv_c),
            "wo": np.ascontiguousarray(woT[g * LF:(g + 1) * LF, :]),
        })
    _CACHE["in_maps"] = in_maps
    res = run_bass_kernel_spmd(nc, in_maps, list(range(8)))

    out = np.empty((B, T, D), np.float32)
    for b in range(B):
        ssq_a = np.zeros((T,), np.float32)
        ysum = np.zeros((T, D), np.float32)
        for g in range(4):
            rr = res.results[b * 4 + g]
            ysum += rr["yp"]
            ssq_a += rr["ssqa"][0]
        ro = 1.0 / np.sqrt(ssq_a / np.float32(D) + EPS)
        out[b] = ysum * ro[:, None]
    return out


if __name__ == "__main__":
    data = np.load("/root/problem/inputs.npz")
    out = kernel(**{k: data[k] for k in data.files})
    ref = np.load("/root/problem/ref_out.npy")
    d = out.astype(np.float64) - ref.astype(np.float64)
    rv = (d * d).sum() / (ref.astype(np.float64) ** 2).sum()
    print("resid_var=%.3e relerr=%.3e absmax=%.3g" %
          (rv, np.sqrt(rv), np.abs(d).max()))
